# revision 11
# baseline (speedup 1.0000x reference)
import numpy as np

_CACHE = {}

N_CORES = 8
TOK = 16384
TOK_PER = TOK // N_CORES  # 2048 tokens per core
DIM = 2048
NE = 64
TOPK = 8
KC = 128            # contraction chunk (partition dim)
NK = DIM // KC      # 16 chunks
NT = 256            # token tile (PSUM bank half)
NJ = TOK_PER // NT  # 4 token tiles
XW = 512            # tokens per x DMA tile (multiple of NT)
N_DMA_ENG = 3       # engines issuing x loads


def _build():
    import concourse.bass as bass
    import concourse.tile as tile
    from concourse import bacc, mybir

    nc = bacc.Bacc(
        "TRN2",
        target_bir_lowering=False,
        debug=False,
        enable_asserts=False,
        num_devices=N_CORES,
    )
    xT = nc.dram_tensor("xT", (DIM, TOK_PER), mybir.dt.float32r, kind="ExternalInput").ap()
    # W packed on host as [KC, NK*NE]: column block k holds W-chunk k transposed
    wP = nc.dram_tensor("WP", (KC, NK * NE), mybir.dt.float32r, kind="ExternalInput").ap()
    out = nc.dram_tensor("logitsT", (NE, TOK_PER), mybir.dt.float32, kind="ExternalOutput").ap()

    f32r = mybir.dt.float32r

    with tile.TileContext(nc) as tc:
        with (
            tc.tile_pool(name="xpool", bufs=(TOK_PER // XW) * NK) as xpool,
            tc.tile_pool(name="wpool", bufs=1) as wpool,
            tc.tile_pool(name="opool", bufs=NJ) as opool,
            tc.tile_pool(name="psum", bufs=NJ, space=bass.MemorySpace.PSUM) as psum,
        ):
            wt = wpool.tile([KC, NK * NE], f32r)
            nc.sync.dma_start(wt[:], wP[:, :])
            # x tiles loaded token-group-major so each group finishes its
            # chunks (and can store its logits) while later groups still
            # stream in. DMAs round-robin over several issuing engines: each
            # dma_start occupies its sequencer for the DGE setup, so one
            # engine alone can't keep the DMA fabric saturated.
            dma_engines = [nc.gpsimd, nc.sync, nc.scalar][:N_DMA_ENG]
            ng = TOK_PER // XW
            xts = [[None] * NK for _ in range(ng)]
            n = 0
            for g in range(ng):
                for k in range(NK):
                    xt = xpool.tile([KC, XW], f32r)
                    dma_engines[n % len(dma_engines)].dma_start(
                        xt[:], xT[k * KC:(k + 1) * KC, g * XW:(g + 1) * XW]
                    )
                    n += 1
                    xts[g][k] = xt
            for j in range(NJ):
                g, o = (j * NT) // XW, (j * NT) % XW
                acc = psum.tile([NE, NT], mybir.dt.float32)
                for k in range(NK):
                    nc.tensor.matmul(
                        acc[:],
                        wt[:, k * NE:(k + 1) * NE],
                        xts[g][k][:, o:o + NT],
                        start=(k == 0),
                        stop=(k == NK - 1),
                    )
                ot = opool.tile([NE, NT], mybir.dt.float32)
                nc.vector.tensor_copy(ot[:], acc[:])
                dma_engines[j % len(dma_engines)].dma_start(
                    out[:, j * NT:(j + 1) * NT], ot[:]
                )
    nc.compile()
    return nc


def _pack_w(W):
    # [KC, NK*NE] with column block k = W[:, k*KC:(k+1)*KC].T
    return np.ascontiguousarray(
        W.T.reshape(NK, KC, NE).transpose(1, 0, 2).reshape(KC, NK * NE)
    )


def kernel(x, W):
    from concourse import bass_utils

    x = np.asarray(x, dtype=np.float32)
    W = np.asarray(W, dtype=np.float32)
    if "nc" not in _CACHE:
        _CACHE["nc"] = _build()
    nc = _CACHE["nc"]

    WP = _pack_w(W)
    in_maps = []
    for i in range(N_CORES):
        xs = x[i * TOK_PER:(i + 1) * TOK_PER]
        in_maps.append({"xT": np.ascontiguousarray(xs.T), "WP": WP})
    res = bass_utils.run_bass_kernel_spmd(nc, in_maps, list(range(N_CORES)))
    logits = np.concatenate(
        [np.asarray(r["logitsT"]).T for r in res.results], axis=0
    ).astype(np.float32)

    m = logits.max(axis=-1, keepdims=True)
    e = np.exp(logits - m)
    scores = e / e.sum(axis=-1, keepdims=True)
    idx = np.argsort(-scores, axis=-1, kind="stable")[:, :TOPK].astype(np.int32)
    w = np.take_along_axis(scores, idx, axis=-1).astype(np.float32)

    # fp32r matmul perturbs logits by ~1e-3 absolute at most; where the
    # top-k ordering is decided by a margin of that scale, re-derive those
    # tokens' scores at full precision so the selected indices match an
    # fp32 computation exactly.
    srt = -np.sort(-scores, axis=-1)[:, :TOPK + 1]
    margin = (srt[:, :-1] - srt[:, 1:]) / np.maximum(srt[:, :-1], 1e-30)
    close = (margin < 1e-2).any(axis=-1)
    if close.any():
        t = np.where(close)[0]
        lg = x[t].astype(np.float64) @ W.astype(np.float64).T
        lg -= lg.max(axis=-1, keepdims=True)
        ee = np.exp(lg)
        sc = ee / ee.sum(axis=-1, keepdims=True)
        ix = np.argsort(-sc, axis=-1, kind="stable")[:, :TOPK].astype(np.int32)
        idx[t] = ix
        w[t] = np.take_along_axis(sc, ix, axis=-1).astype(np.float32)
    return w, idx
